# revision 2
# baseline (speedup 1.0000x reference)
"""CRF loss (dense Gaussian bilateral filter) on 8 Trainium2 NeuronCores.

Math: with feats f_i (coords/ALPHA ++ I/BETA), K[i,j] = exp(-0.5*||f_i-f_j||^2),
s = K @ 1, n = (s+EPS)^-1/2, H = softmax(U), v_c = n*H_c:
    loss = n^T K n - sum_c v_c^T K v_c
(uses sum_c H_c = 1; per-batch block-diagonal K).

Sharding: cores 0-3 -> batch 0, cores 4-7 -> batch 1. Within a batch, each
core owns a 1536-wide slice of the (6144-padded) output-row space and the
full contraction over all 5888-padded j. K tiles are computed once
(PE matmul for dot products in 3-way-split bf16, ScalarE exp) and kept in
SBUF as bf16 [j-part 128, i-free 1536] tiles. Row sums come FREE from the
EXP pass: activation accum_out gives per-j-partition partial sums over the
core's own i slice ([128, 46] f32); an AllReduce across the 4-core batch
group sums the 4 i-slices into the full s (already in the [j-part, j-blk]
layout the W build wants). The 5-channel weighted filter (n, n*H_c) then
streams the stored K tiles through the PE once more (even jb: W stationary;
odd jb: E stationary through the LDWEIGHTS port, output transposed). The
scalar loss is reduced on host from per-core [5,1536] filter outputs + s.
"""

import numpy as np
import ml_dtypes

import concourse.bass as bass
import concourse.bacc as bacc
import concourse.tile as tile
import concourse.mybir as mybir
import concourse.bass_utils as bass_utils
from concourse.hw_specs import get_activation_tables

ALPHA = 5.0
BETA = 5.0
EPS = 1e-20

B = 2
C = 4
XD = YD = ZD = 18
N = XD * YD * ZD          # 5832
NJ = 5888                 # j padded to 46*128
NJB = 46                  # j blocks of 128
IW = 1536                 # i-rows per core (12*128)
NIC = IW // 128           # 12 i chunks per core
NEG = -120.0              # pad bias => exp -> 0

F32 = mybir.dt.float32
BF16 = mybir.dt.bfloat16

# Collective variant: "2g" = AllReduce in two 4-core groups;
# "8g" = single 8-core AllReduce on a group-masked [2, 128, 46] buffer.
COLL = "2g"

TRACE = False
LAST_RESULT = None

_compiled = {}


def _build():
    nc = bacc.Bacc("TRN2", target_bir_lowering=False, debug=False, num_devices=8)

    fhat = nc.dram_tensor("fhat", [39, NJ], BF16, kind="ExternalInput")
    frhs = nc.dram_tensor("frhs", [39, IW], BF16, kind="ExternalInput")
    biasj = nc.dram_tensor("biasj", [128, NJB], F32, kind="ExternalInput")
    h1 = nc.dram_tensor("h1", [128, C * NJB], F32, kind="ExternalInput")
    gmask = nc.dram_tensor("gmask", [128, 2], F32, kind="ExternalInput")
    out = nc.dram_tensor("out", [5, IW], F32, kind="ExternalOutput")
    yt = nc.dram_tensor("yt", [128, 5 * NIC], F32, kind="ExternalOutput")
    sout = nc.dram_tensor("sout", [128, NJB], F32, kind="ExternalOutput")

    with tile.TileContext(nc) as tc:
        with (
            tc.tile_pool(name="const", bufs=1) as cp,
            tc.tile_pool(name="epool", bufs=1) as ep,
            tc.tile_pool(name="dram", bufs=1, space="DRAM") as dp,
        ):
            fhat_sb = cp.tile([39, NJ], BF16)
            frhs_sb = cp.tile([39, IW], BF16)
            bias_sb = cp.tile([128, NJB], F32)
            h1_sb = cp.tile([128, C * NJB], F32)
            gm_sb = cp.tile([128, 2], F32)
            eps_sb = cp.tile([128, 1], F32)
            spart_sb = cp.tile([128, NJB], F32)
            ssb = cp.tile([128, NJB], F32)
            lnsb = cp.tile([128, NJB], F32)
            nsb = cp.tile([128, NJB], F32)
            w_sb = cp.tile([128, 5 * NJB], BF16)
            y_sb = cp.tile([5, IW], F32)
            yt_sb = cp.tile([128, 5 * NIC], F32)
            e_sb = ep.tile([128, NJB * IW], BF16)
            if COLL == "8g":
                stage_sb = cp.tile([128, 2 * NJB], F32)
                gath_sb = cp.tile([128, 2 * NJB], F32)
                tmp_sb = cp.tile([128, NJB], F32)
                bounce = dp.tile([2 * 128 * NJB], F32)
                gath = dp.tile([2 * 128 * NJB], F32)
            else:
                bounce = dp.tile([128 * NJB], F32)
                gath = dp.tile([128 * NJB], F32)

            nc.sync.dma_start(fhat_sb[:, 0:512], fhat[:, 0:512])
            for ch in range(3):
                nc.sync.dma_start(
                    frhs_sb[:, 512 * ch : 512 * (ch + 1)],
                    frhs[:, 512 * ch : 512 * (ch + 1)],
                )
            nc.sync.dma_start(bias_sb[:], biasj[:])
            for a in range(512, NJ, 1536):
                b = min(a + 1536, NJ)
                nc.sync.dma_start(fhat_sb[:, a:b], fhat[:, a:b])
            nc.sync.dma_start(h1_sb[:], h1[:])
            nc.sync.dma_start(gm_sb[:], gmask[:])
            nc.vector.memset(eps_sb[:], EPS)

            # Preload the table set holding BOTH Exp and Ln so no ACT table
            # switches land on the critical path mid-kernel.
            _tabs = list(get_activation_tables("gen3"))
            _nlx = _tabs.index("natural_log_exp_and_others")
            nc.scalar.add_instruction(
                mybir.InstLoadActFuncSet(
                    name=f"I-{nc.next_id()}", act_func_set_id=_nlx
                )
            )

            # ---- pass A: dot -> exp (accum_out = per-j partial row sums) ----
            with tc.tile_pool(name="dotp", bufs=2, space="PSUM") as dotp:
                for jb in range(NJB):
                    dps = dotp.tile([128, IW], F32, tag="dot")
                    lw = fhat_sb[:, 128 * jb : 128 * (jb + 1)]
                    for ch in range(3):
                        nc.tensor.matmul(
                            dps[:, 512 * ch : 512 * (ch + 1)],
                            lw,
                            frhs_sb[:, 512 * ch : 512 * (ch + 1)],
                            start=True,
                            stop=True,
                        )
                    nc.scalar.activation(
                        e_sb[:, IW * jb : IW * (jb + 1)],
                        dps[:, :],
                        mybir.ActivationFunctionType.Exp,
                        bias=bias_sb[:, jb : jb + 1],
                        scale=1.0,
                        accum_out=spart_sb[:, jb : jb + 1],
                    )

            # ---- AllReduce the partial s across the 4-core batch group ----
            if COLL == "8g":
                # SPMD-uniform: write own partial into own batch's half (via
                # the per-core one-hot gmask), zeros into the other; a single
                # 8-rank AllReduce then produces both batches' s everywhere.
                for h in range(2):
                    nc.vector.tensor_scalar_mul(
                        stage_sb[:, NJB * h : NJB * (h + 1)],
                        spart_sb[:, :],
                        gm_sb[:, h : h + 1],
                    )
                nc.sync.dma_start(
                    bounce[:].rearrange("(h p j) -> p h j", p=128, j=NJB),
                    stage_sb[:, :].rearrange("p (h j) -> p h j", j=NJB),
                )
                nc.gpsimd.collective_compute(
                    "AllReduce",
                    mybir.AluOpType.add,
                    replica_groups=[[0, 1, 2, 3, 4, 5, 6, 7]],
                    ins=[bounce[:]],
                    outs=[gath[:]],
                )
                nc.sync.dma_start(
                    gath_sb[:, :].rearrange("p (h j) -> p h j", j=NJB),
                    gath[:].rearrange("(h p j) -> p h j", p=128, j=NJB),
                )
                nc.vector.tensor_scalar_mul(
                    ssb[:, :], gath_sb[:, 0:NJB], gm_sb[:, 0:1]
                )
                nc.vector.tensor_scalar_mul(
                    tmp_sb[:, :], gath_sb[:, NJB : 2 * NJB], gm_sb[:, 1:2]
                )
                nc.vector.tensor_add(ssb[:, :], ssb[:, :], tmp_sb[:, :])
            else:
                nc.sync.dma_start(
                    bounce[:].rearrange("(p j) -> p j", j=NJB), spart_sb[:, :]
                )
                nc.gpsimd.collective_compute(
                    "AllReduce",
                    mybir.AluOpType.add,
                    replica_groups=[[0, 1, 2, 3], [4, 5, 6, 7]],
                    ins=[bounce[:]],
                    outs=[gath[:]],
                )
                nc.sync.dma_start(
                    ssb[:, :], gath[:].rearrange("(p j) -> p j", j=NJB)
                )

            # n = (s + EPS)^-1/2 via exp(-0.5 * ln(s + EPS))
            nc.scalar.activation(
                lnsb[:, :],
                ssb[:, :],
                mybir.ActivationFunctionType.Ln,
                bias=eps_sb[:, 0:1],
                scale=1.0,
            )
            nc.scalar.activation(
                nsb[:, :],
                lnsb[:, :],
                mybir.ActivationFunctionType.Exp,
                scale=-0.5,
            )

            # ---- W planes: [n, n*H_0..n*H_3] in bf16, plane-major ----
            nc.vector.tensor_copy(w_sb[:, 0:NJB], nsb[:, 0:NJB])
            for c in range(C):
                nc.vector.tensor_mul(
                    w_sb[:, NJB * (c + 1) : NJB * (c + 2)],
                    nsb[:, 0:NJB],
                    h1_sb[:, NJB * c : NJB * (c + 1)],
                )
            nc.sync.dma_start(sout[:, :], ssb[:, :])

            # ---- pass B: Y[5, IW] = W^T E accumulated over all j blocks.
            # Even jb stream E through the PE rhs port (W stationary); odd jb
            # load E chunks through the LDWEIGHTS port and stream the tiny W
            # (output transposed, [i-chunk, 5] per chunk). The two forms use
            # the PE's two independent SBUF read ports, nearly halving the
            # streamed column count on the critical rhs path. Host adds the
            # transposed half back in.
            w_view = w_sb[:, :].rearrange("p (r j) -> p r j", j=NJB)
            evens = [jb for jb in range(NJB) if jb % 3 == 0]
            odds = [jb for jb in range(NJB) if jb % 3 != 0]
            with tc.tile_pool(name="ypool", bufs=1, space="PSUM") as yp:
                y_ps = yp.tile([5, IW], F32)
                yt_ps = yp.tile([128, 5 * NIC], F32)
                nc.vector.memset(yt_ps[:, :], 0.0)
                ne_seen = no_seen = 0
                for jb in range(NJB):
                    if jb in evens:
                        ne_seen += 1
                        lw = w_view[:, :, jb]
                        for ch in range(3):
                            nc.tensor.matmul(
                                y_ps[:, 512 * ch : 512 * (ch + 1)],
                                lw,
                                e_sb[:, IW * jb + 512 * ch : IW * jb + 512 * (ch + 1)],
                                start=(ne_seen == 1),
                                stop=(ne_seen == len(evens)),
                            )
                    else:
                        no_seen += 1
                        for m in range(NIC):
                            nc.tensor.matmul(
                                yt_ps[:, 5 * m : 5 * (m + 1)],
                                e_sb[:, IW * jb + 128 * m : IW * jb + 128 * (m + 1)],
                                w_view[:, :, jb],
                                start=False,
                                stop=(no_seen == len(odds)),
                                skip_group_check=True,
                            )
                nc.vector.tensor_copy(y_sb[:, :], y_ps[:, :])
                nc.vector.tensor_copy(yt_sb[:, :], yt_ps[:, :])
                nc.sync.dma_start(out[0:5, :], y_sb[:, :])
                nc.sync.dma_start(yt[:, :], yt_sb[:, :])

    nc.compile()
    return nc


def _split3(a):
    """3-way bf16 split: a ~ h + m + l to ~24 mantissa bits."""
    bf = ml_dtypes.bfloat16
    h = a.astype(bf)
    r1 = a - h.astype(np.float32)
    m = r1.astype(bf)
    l = (r1 - m.astype(np.float32)).astype(bf)
    return h, m, l


def kernel(I, U):
    global LAST_RESULT
    if "nc" not in _compiled:
        _compiled["nc"] = _build()
    nc = _compiled["nc"]

    I = np.asarray(I, np.float32)
    U = np.asarray(U, np.float32)

    g = np.arange(XD, dtype=np.float32)
    gx, gy, gz = np.meshgrid(g, g, g, indexing="ij")
    coords = np.stack([gx, gy, gz], 0).reshape(3, N)

    in_maps = []
    host = []
    for k in range(8):
        b, r = divmod(k, 4)
        feats = np.concatenate(
            [coords / ALPHA, I[b].reshape(3, N) / BETA], 0
        ).astype(np.float32)  # [6, N]
        sq = (feats.astype(np.float64) ** 2).sum(0)  # [N] f64
        shalf = (-0.5 * sq).astype(np.float32)
        bf = ml_dtypes.bfloat16
        fh, fm, fl = _split3(feats)
        s1, s2, s3 = _split3(shalf)

        one = np.ones((1, N), bf)
        fhat = np.zeros((39, NJ), bf)
        fhat[:, :N] = np.concatenate([fh, fh, fm, fh, fl, fm, one, one, one], 0)

        gi = IW * r + np.arange(IW)
        valid = gi < N
        giv = gi[valid]
        frhs = np.zeros((39, IW), bf)
        frhs[:, valid] = np.concatenate(
            [
                fh[:, giv], fm[:, giv], fh[:, giv], fl[:, giv], fh[:, giv],
                fm[:, giv], s1[None, giv], s2[None, giv], s3[None, giv],
            ],
            0,
        )
        frhs[36, ~valid] = bf(NEG)

        bpad = np.full(NJ, NEG, np.float32)
        bpad[:N] = shalf
        biasj = bpad.reshape(NJB, 128).T.copy()  # [128, NJB]

        uf = U[b].reshape(C, N).astype(np.float64)
        uf = uf - uf.max(0, keepdims=True)
        e = np.exp(uf)
        H1 = (e / e.sum(0, keepdims=True)).astype(np.float32)  # [C, N]
        hpad = np.zeros((C, NJ), np.float32)
        hpad[:, :N] = H1
        h1in = np.concatenate(
            [hpad[c].reshape(NJB, 128).T for c in range(C)], axis=1
        ).copy()  # [128, C*NJB]

        gmask = np.zeros((128, 2), np.float32)
        gmask[:, b] = 1.0

        in_maps.append(
            {"fhat": fhat, "frhs": frhs, "biasj": biasj, "h1": h1in,
             "gmask": gmask}
        )
        host.append((H1, valid, giv, gi))

    res = bass_utils.run_bass_kernel_spmd(
        nc, in_maps, core_ids=list(range(8)), trace=TRACE
    )
    LAST_RESULT = res

    loss = 0.0
    for k in range(8):
        b, r = divmod(k, 4)
        H1, valid, giv, gi = host[k]
        o = res.results[k]["out"].astype(np.float64)
        ytk = res.results[k]["yt"].astype(np.float64)  # [128, 5*NIC]
        sfull = res.results[k]["sout"].astype(np.float64)  # [128, NJB]
        # yt[p, 5m+r] = Y_odd[r, 128m+p]
        yodd = ytk.reshape(128, NIC, 5).transpose(2, 1, 0).reshape(5, IW)
        yfull = o[0:5] + yodd
        yv = yfull[:, valid]  # [5, nvalid]
        # s for own i rows: global voxel i -> (partition i%128, block i//128)
        s = sfull[giv % 128, giv // 128]
        n = 1.0 / np.sqrt(s + EPS)
        hv = H1[:, giv].astype(np.float64)  # [C, nvalid]
        loss += (n * yv[0]).sum()
        for c in range(C):
            loss -= (n * hv[c] * yv[1 + c]).sum()
    return np.float32(loss)


# revision 5
# speedup vs baseline: 1.3563x; 1.3563x over previous
"""CRF loss (dense Gaussian bilateral filter) on 8 Trainium2 NeuronCores.

Math: with feats f_i (coords/ALPHA ++ I/BETA), K[i,j] = exp(-0.5*||f_i-f_j||^2),
s = K @ 1, n = (s+EPS)^-1/2, H = softmax(U), v_c = n*H_c:
    loss = n^T K n - sum_c v_c^T K v_c
(uses sum_c H_c = 1; per-batch block-diagonal K).

Sharding: cores 0-3 -> batch 0, cores 4-7 -> batch 1. Within a batch, each
core owns a 1536-wide slice of the (6144-padded) output-row space and the
full contraction over all 5888-padded j. K tiles are computed once
(PE matmul for dot products in 3-way-split bf16, ScalarE exp) and kept in
SBUF as bf16 [j-part 128, i-free 1536] tiles. Row sums come FREE from the
EXP pass: activation accum_out gives per-j-partition partial sums over the
core's own i slice ([128, 46] f32); an AllReduce across the 4-core batch
group sums the 4 i-slices into the full s (already in the [j-part, j-blk]
layout the W build wants). The 5-channel weighted filter (n, n*H_c) then
streams the stored K tiles through the PE once more (even jb: W stationary;
odd jb: E stationary through the LDWEIGHTS port, output transposed). The
scalar loss is reduced on host from per-core [5,1536] filter outputs + s.
"""

import numpy as np
import ml_dtypes

import concourse.bass as bass
import concourse.bacc as bacc
import concourse.tile as tile
import concourse.mybir as mybir
import concourse.bass_utils as bass_utils
from concourse.hw_specs import get_activation_tables

ALPHA = 5.0
BETA = 5.0
EPS = 1e-20

B = 2
C = 4
XD = YD = ZD = 18
N = XD * YD * ZD          # 5832
NJ = 5888                 # j padded to 46*128
NJB = 46                  # j blocks of 128
IW = 1536                 # i-rows per core (12*128)
NIC = IW // 128           # 12 i chunks per core
NEG = -120.0              # pad bias => exp -> 0

F32 = mybir.dt.float32
BF16 = mybir.dt.bfloat16

# Collective variant: "ag" = AllGather partials in two 4-core groups then
# sum on DVE; "2g" = AllReduce in two 4-core groups; "8g" = single 8-core
# AllReduce on a group-masked [2, 128, 46] buffer.
COLL = "ag"

TRACE = False
LAST_RESULT = None

_compiled = {}


def _build():
    nc = bacc.Bacc("TRN2", target_bir_lowering=False, debug=False, num_devices=8)

    fhat = nc.dram_tensor("fhat", [39, NJ], BF16, kind="ExternalInput")
    frhs = nc.dram_tensor("frhs", [39, IW], BF16, kind="ExternalInput")
    biasj = nc.dram_tensor("biasj", [128, NJB], F32, kind="ExternalInput")
    h1 = nc.dram_tensor("h1", [128, C * NJB], F32, kind="ExternalInput")
    gmask = nc.dram_tensor("gmask", [128, 2], F32, kind="ExternalInput")
    out = nc.dram_tensor("out", [5, IW], F32, kind="ExternalOutput")
    yt = nc.dram_tensor("yt", [128, 5 * NIC], F32, kind="ExternalOutput")
    sout = nc.dram_tensor("sout", [128, NJB], F32, kind="ExternalOutput")

    with tile.TileContext(nc) as tc:
        with (
            tc.tile_pool(name="const", bufs=1) as cp,
            tc.tile_pool(name="epool", bufs=1) as ep,
            tc.tile_pool(name="dram", bufs=1, space="DRAM") as dp,
        ):
            fhat_sb = cp.tile([39, NJ], BF16)
            frhs_sb = cp.tile([39, IW], BF16)
            bias_sb = cp.tile([128, NJB], F32)
            h1_sb = cp.tile([128, C * NJB], F32)
            gm_sb = cp.tile([128, 2], F32)
            eps_sb = cp.tile([128, 1], F32)
            spart_sb = cp.tile([128, NJB], F32)
            ssb = cp.tile([128, NJB], F32)
            lnsb = cp.tile([128, NJB], F32)
            nsb = cp.tile([128, NJB], F32)
            w_sb = cp.tile([128, 5 * NJB], BF16)
            y_sb = cp.tile([5, IW], F32)
            yt_sb = cp.tile([128, 5 * NIC], F32)
            e_sb = ep.tile([128, NJB * IW], BF16)
            if COLL == "8g":
                stage_sb = cp.tile([128, 2 * NJB], F32)
                gath_sb = cp.tile([128, 2 * NJB], F32)
                tmp_sb = cp.tile([128, NJB], F32)
                bounce = dp.tile([2 * 128 * NJB], F32)
                gath = dp.tile([2 * 128 * NJB], F32)
            elif COLL == "ag":
                gath_sb = cp.tile([128, 4 * NJB], F32)
                bounce = dp.tile([128 * NJB], F32)
                gath = dp.tile([4 * 128 * NJB], F32)
            else:
                bounce = dp.tile([128 * NJB], F32)
                gath = dp.tile([128 * NJB], F32)

            nc.sync.dma_start(fhat_sb[:, 0:512], fhat[:, 0:512])
            for ch in range(3):
                nc.sync.dma_start(
                    frhs_sb[:, 512 * ch : 512 * (ch + 1)],
                    frhs[:, 512 * ch : 512 * (ch + 1)],
                )
            nc.sync.dma_start(bias_sb[:], biasj[:])
            for a in range(512, NJ, 1536):
                b = min(a + 1536, NJ)
                nc.sync.dma_start(fhat_sb[:, a:b], fhat[:, a:b])
            nc.sync.dma_start(h1_sb[:], h1[:])
            nc.sync.dma_start(gm_sb[:], gmask[:])
            nc.vector.memset(eps_sb[:], EPS)

            # Preload the table set holding BOTH Exp and Ln so no ACT table
            # switches land on the critical path mid-kernel.
            _tabs = list(get_activation_tables("gen3"))
            _nlx = _tabs.index("natural_log_exp_and_others")
            nc.scalar.add_instruction(
                mybir.InstLoadActFuncSet(
                    name=f"I-{nc.next_id()}", act_func_set_id=_nlx
                )
            )

            # ---- pass A: dot -> exp (accum_out = per-j partial row sums) ----
            with tc.tile_pool(name="dotp", bufs=2, space="PSUM") as dotp:
                for jb in range(NJB):
                    dps = dotp.tile([128, IW], F32, tag="dot")
                    lw = fhat_sb[:, 128 * jb : 128 * (jb + 1)]
                    for ch in range(3):
                        nc.tensor.matmul(
                            dps[:, 512 * ch : 512 * (ch + 1)],
                            lw,
                            frhs_sb[:, 512 * ch : 512 * (ch + 1)],
                            start=True,
                            stop=True,
                        )
                    nc.scalar.activation(
                        e_sb[:, IW * jb : IW * (jb + 1)],
                        dps[:, :],
                        mybir.ActivationFunctionType.Exp,
                        bias=bias_sb[:, jb : jb + 1],
                        scale=1.0,
                        accum_out=spart_sb[:, jb : jb + 1],
                    )

            # ---- AllReduce the partial s across the 4-core batch group ----
            if COLL == "8g":
                # SPMD-uniform: write own partial into own batch's half (via
                # the per-core one-hot gmask), zeros into the other; a single
                # 8-rank AllReduce then produces both batches' s everywhere.
                for h in range(2):
                    nc.vector.tensor_scalar_mul(
                        stage_sb[:, NJB * h : NJB * (h + 1)],
                        spart_sb[:, :],
                        gm_sb[:, h : h + 1],
                    )
                nc.sync.dma_start(
                    bounce[:].rearrange("(h p j) -> p h j", p=128, j=NJB),
                    stage_sb[:, :].rearrange("p (h j) -> p h j", j=NJB),
                )
                nc.gpsimd.collective_compute(
                    "AllReduce",
                    mybir.AluOpType.add,
                    replica_groups=[[0, 1, 2, 3, 4, 5, 6, 7]],
                    ins=[bounce[:]],
                    outs=[gath[:]],
                )
                nc.sync.dma_start(
                    gath_sb[:, :].rearrange("p (h j) -> p h j", j=NJB),
                    gath[:].rearrange("(h p j) -> p h j", p=128, j=NJB),
                )
                nc.vector.tensor_scalar_mul(
                    ssb[:, :], gath_sb[:, 0:NJB], gm_sb[:, 0:1]
                )
                nc.vector.tensor_scalar_mul(
                    tmp_sb[:, :], gath_sb[:, NJB : 2 * NJB], gm_sb[:, 1:2]
                )
                nc.vector.tensor_add(ssb[:, :], ssb[:, :], tmp_sb[:, :])
            elif COLL == "ag":
                nc.sync.dma_start(
                    bounce[:].rearrange("(p j) -> p j", j=NJB), spart_sb[:, :]
                )
                nc.gpsimd.collective_compute(
                    "AllGather",
                    mybir.AluOpType.bypass,
                    replica_groups=[[0, 1, 2, 3], [4, 5, 6, 7]],
                    ins=[bounce[:]],
                    outs=[gath[:]],
                )
                nc.sync.dma_start(
                    gath_sb[:, :].rearrange("p (r j) -> p r j", j=NJB),
                    gath[:].rearrange("(r p j) -> p r j", p=128, j=NJB),
                )
                nc.vector.tensor_add(
                    ssb[:, :], gath_sb[:, 0:NJB], gath_sb[:, NJB : 2 * NJB]
                )
                nc.vector.tensor_add(
                    lnsb[:, :],
                    gath_sb[:, 2 * NJB : 3 * NJB],
                    gath_sb[:, 3 * NJB : 4 * NJB],
                )
                nc.vector.tensor_add(ssb[:, :], ssb[:, :], lnsb[:, :])
            else:
                nc.sync.dma_start(
                    bounce[:].rearrange("(p j) -> p j", j=NJB), spart_sb[:, :]
                )
                nc.gpsimd.collective_compute(
                    "AllReduce",
                    mybir.AluOpType.add,
                    replica_groups=[[0, 1, 2, 3], [4, 5, 6, 7]],
                    ins=[bounce[:]],
                    outs=[gath[:]],
                )
                nc.sync.dma_start(
                    ssb[:, :], gath[:].rearrange("(p j) -> p j", j=NJB)
                )

            # n = (s + EPS)^-1/2 via exp(-0.5 * ln(s + EPS))
            nc.scalar.activation(
                lnsb[:, :],
                ssb[:, :],
                mybir.ActivationFunctionType.Ln,
                bias=eps_sb[:, 0:1],
                scale=1.0,
            )
            nc.scalar.activation(
                nsb[:, :],
                lnsb[:, :],
                mybir.ActivationFunctionType.Exp,
                scale=-0.5,
            )

            # ---- W planes: [n, n*H_0..n*H_3] in bf16, plane-major ----
            nc.vector.tensor_copy(w_sb[:, 0:NJB], nsb[:, 0:NJB])
            for c in range(C):
                nc.vector.tensor_mul(
                    w_sb[:, NJB * (c + 1) : NJB * (c + 2)],
                    nsb[:, 0:NJB],
                    h1_sb[:, NJB * c : NJB * (c + 1)],
                )
            nc.sync.dma_start(sout[:, :], ssb[:, :])

            # ---- pass B: Y[5, IW] = W^T E accumulated over all j blocks.
            # Even jb stream E through the PE rhs port (W stationary); odd jb
            # load E chunks through the LDWEIGHTS port and stream the tiny W
            # (output transposed, [i-chunk, 5] per chunk). The two forms use
            # the PE's two independent SBUF read ports, nearly halving the
            # streamed column count on the critical rhs path. Host adds the
            # transposed half back in.
            w_view = w_sb[:, :].rearrange("p (r j) -> p r j", j=NJB)
            evens = [jb for jb in range(NJB) if jb % 3 == 0]
            odds = [jb for jb in range(NJB) if jb % 3 != 0]
            with tc.tile_pool(name="ypool", bufs=1, space="PSUM") as yp:
                y_ps = yp.tile([5, IW], F32)
                yt_ps = yp.tile([128, 5 * NIC], F32)
                nc.vector.memset(yt_ps[:, :], 0.0)
                ne_seen = no_seen = 0
                for jb in range(NJB):
                    if jb in evens:
                        ne_seen += 1
                        lw = w_view[:, :, jb]
                        for ch in range(3):
                            nc.tensor.matmul(
                                y_ps[:, 512 * ch : 512 * (ch + 1)],
                                lw,
                                e_sb[:, IW * jb + 512 * ch : IW * jb + 512 * (ch + 1)],
                                start=(ne_seen == 1),
                                stop=(ne_seen == len(evens)),
                            )
                    else:
                        no_seen += 1
                        for m in range(NIC):
                            nc.tensor.matmul(
                                yt_ps[:, 5 * m : 5 * (m + 1)],
                                e_sb[:, IW * jb + 128 * m : IW * jb + 128 * (m + 1)],
                                w_view[:, :, jb],
                                start=False,
                                stop=(no_seen == len(odds)),
                                skip_group_check=True,
                            )
                nc.vector.tensor_copy(y_sb[:, :], y_ps[:, :])
                nc.vector.tensor_copy(yt_sb[:, :], yt_ps[:, :])
                nc.sync.dma_start(out[0:5, :], y_sb[:, :])
                nc.sync.dma_start(yt[:, :], yt_sb[:, :])

    nc.compile()
    return nc


def _split3(a):
    """3-way bf16 split: a ~ h + m + l to ~24 mantissa bits."""
    bf = ml_dtypes.bfloat16
    h = a.astype(bf)
    r1 = a - h.astype(np.float32)
    m = r1.astype(bf)
    l = (r1 - m.astype(np.float32)).astype(bf)
    return h, m, l


def kernel(I, U):
    global LAST_RESULT
    if "nc" not in _compiled:
        _compiled["nc"] = _build()
    nc = _compiled["nc"]

    I = np.asarray(I, np.float32)
    U = np.asarray(U, np.float32)

    g = np.arange(XD, dtype=np.float32)
    gx, gy, gz = np.meshgrid(g, g, g, indexing="ij")
    coords = np.stack([gx, gy, gz], 0).reshape(3, N)

    in_maps = []
    host = []
    for k in range(8):
        b, r = divmod(k, 4)
        feats = np.concatenate(
            [coords / ALPHA, I[b].reshape(3, N) / BETA], 0
        ).astype(np.float32)  # [6, N]
        sq = (feats.astype(np.float64) ** 2).sum(0)  # [N] f64
        shalf = (-0.5 * sq).astype(np.float32)
        bf = ml_dtypes.bfloat16
        fh, fm, fl = _split3(feats)
        s1, s2, s3 = _split3(shalf)

        one = np.ones((1, N), bf)
        fhat = np.zeros((39, NJ), bf)
        fhat[:, :N] = np.concatenate([fh, fh, fm, fh, fl, fm, one, one, one], 0)

        gi = IW * r + np.arange(IW)
        valid = gi < N
        giv = gi[valid]
        frhs = np.zeros((39, IW), bf)
        frhs[:, valid] = np.concatenate(
            [
                fh[:, giv], fm[:, giv], fh[:, giv], fl[:, giv], fh[:, giv],
                fm[:, giv], s1[None, giv], s2[None, giv], s3[None, giv],
            ],
            0,
        )
        frhs[36, ~valid] = bf(NEG)

        bpad = np.full(NJ, NEG, np.float32)
        bpad[:N] = shalf
        biasj = bpad.reshape(NJB, 128).T.copy()  # [128, NJB]

        uf = U[b].reshape(C, N).astype(np.float64)
        uf = uf - uf.max(0, keepdims=True)
        e = np.exp(uf)
        H1 = (e / e.sum(0, keepdims=True)).astype(np.float32)  # [C, N]
        hpad = np.zeros((C, NJ), np.float32)
        hpad[:, :N] = H1
        h1in = np.concatenate(
            [hpad[c].reshape(NJB, 128).T for c in range(C)], axis=1
        ).copy()  # [128, C*NJB]

        gmask = np.zeros((128, 2), np.float32)
        gmask[:, b] = 1.0

        in_maps.append(
            {"fhat": fhat, "frhs": frhs, "biasj": biasj, "h1": h1in,
             "gmask": gmask}
        )
        host.append((H1, valid, giv, gi))

    res = bass_utils.run_bass_kernel_spmd(
        nc, in_maps, core_ids=list(range(8)), trace=TRACE
    )
    LAST_RESULT = res

    loss = 0.0
    for k in range(8):
        b, r = divmod(k, 4)
        H1, valid, giv, gi = host[k]
        o = res.results[k]["out"].astype(np.float64)
        ytk = res.results[k]["yt"].astype(np.float64)  # [128, 5*NIC]
        sfull = res.results[k]["sout"].astype(np.float64)  # [128, NJB]
        # yt[p, 5m+r] = Y_odd[r, 128m+p]
        yodd = ytk.reshape(128, NIC, 5).transpose(2, 1, 0).reshape(5, IW)
        yfull = o[0:5] + yodd
        yv = yfull[:, valid]  # [5, nvalid]
        # s for own i rows: global voxel i -> (partition i%128, block i//128)
        s = sfull[giv % 128, giv // 128]
        n = 1.0 / np.sqrt(s + EPS)
        hv = H1[:, giv].astype(np.float64)  # [C, nvalid]
        loss += (n * yv[0]).sum()
        for c in range(C):
            loss -= (n * hv[c] * yv[1 + c]).sum()
    return np.float32(loss)
